# revision 25
# baseline (speedup 1.0000x reference)
"""Local2DAttention TRN2 kernel (nn_Local2DAttention_79207786873330).

Math (faithful to the reference's torch-bug semantics):
  x (16, 1024, 512) is window-blocked into M=256 "windows" (b, i, j) of 8x8
  spatial positions. A plain row-major reshape of each (E, 8, 8) block into
  (64, 512) scrambles channels/spatial into 64 tokens per window:
      y[m, t, e] = x[b, (i*8+w1)*32 + j*8 + w2, 8t+a],  e = a*64 + w1*8 + w2
  nn.MultiheadAttention (batch_first=False) then attends over the M=256 axis
  with the 64 t-positions as batch and 8 heads:
      per (t, h): S = Q K^T / 8 over 256x256, softmax, O = P V.

Sharding: the 64 t-positions split 8 per core (t = 8*cc + tl). Attention,
projections and output assembly are fully independent per t -> zero
cross-core communication. Weights are replicated.

Device pipeline per core (bf16 matmul operands, fp32 accumulation):
  yT (512, 2048)      - host-permuted token matrix, f-major (e x tokens)
  QK^T proj: PROJ^T[r, tok] = Wqk^T.T @ yT  (q rows pre-scaled by 1/8;
             K bias dropped - softmax is invariant to it)
  V    proj: V[tok, 512]  = yT.T @ Wv  (plain 64-col heads, no ones col)
  per (t-pair tp, head-pair hp):
      scores S^T = K^T.T @ Q^T -> exp (ACT, ->bf16) per (ti, hh)
      D rows via PE "staircase" matmuls: dall[k=(ti,hh), l] = sum_s exp
      AV col-group packed: pso[hh*64+d, ti*256+l] (2 heads concurrent)
      one reciprocal_approx_fast (4, 256) -> invD (f32r)
      PE broadcast invD -> rbps (128, 512), ACT evict -> SBUF
      one DVE mul: ot = pso * rbb  (normalized O^T, bf16)
  out proj: Z = O^T.T @ Wout^T + b_eff  (b_eff = b_out + Wout @ b_v, host)
  output stored bf16, upcast on host.
"""
import sys
sys.path.insert(0, '/opt/trn_rl_repo')
import numpy as np
import ml_dtypes

BF = ml_dtypes.bfloat16

# problem constants (hardcoded per contract)
B, N, E = 16, 1024, 512
WIN = 8          # window_size
HS = 4           # hS = S // W,  S = 32
NH = 8           # heads
HD = 64          # head dim
NCORES = 8
TL = 8           # t-values per core
MTOK = 256       # windows (= B*HS*HS) = tokens per t
TOK = TL * MTOK  # tokens per core

_cache = {}


def _split_multiwaits(nc, mybir, limit=1):
    """This toolchain's walrus encodes at most one semaphore wait per
    instruction; hoist excess waits into preceding NoOps on the same engine."""
    n_split = 0
    for f in nc.m.functions:
        for blk in f.blocks:
            insts = blk.instructions
            out = []
            for inst in insts:
                si = inst.sync_info
                waits = list(si.on_wait) if (si is not None and si.on_wait) else []
                if len(waits) > limit:
                    excess, keep = waits[:-limit], waits[-limit:]
                    for w in excess:
                        nop = mybir.InstNoOp(
                            name=f"{inst.name}-wsplit{n_split}",
                            engine=inst.engine,
                            ins=[], outs=[],
                            sync_info=mybir.SyncInfo(on_wait=[w], on_update=[]),
                        )
                        out.append(nop)
                        n_split += 1
                    inst.sync_info = mybir.SyncInfo(
                        on_wait=keep, on_update=list(si.on_update or []))
                out.append(inst)
            if n_split:
                insts.clear()
                insts.extend(out)
    return n_split


def _build_module(split_waits=True):
    import concourse.bass as bass
    import concourse.mybir as mybir
    from concourse import tile

    f32 = mybir.dt.float32
    f32r = mybir.dt.float32r
    bf16 = mybir.dt.bfloat16
    Exp = mybir.ActivationFunctionType.Exp

    nc = bass.Bass()
    YT = nc.dram_tensor("yT", [E, TOK], bf16, kind="ExternalInput")
    WQK = nc.dram_tensor("wqk", [E, 2 * E], bf16, kind="ExternalInput")
    WV = nc.dram_tensor("wv", [E, E], bf16, kind="ExternalInput")
    WO = nc.dram_tensor("wo", [E, E], bf16, kind="ExternalInput")
    BQK = nc.dram_tensor("bqk", [128, 4], f32, kind="ExternalInput")
    BEFF = nc.dram_tensor("beff", [1, E], f32, kind="ExternalInput")
    STAIR = nc.dram_tensor("stair", [128, 8], bf16, kind="ExternalInput")
    SEL4 = nc.dram_tensor("sel4", [4, 256], bf16, kind="ExternalInput")
    OUT = nc.dram_tensor("o", [TOK, E], bf16, kind="ExternalOutput")

    with tile.TileContext(nc) as tc:
        with (
            tc.tile_pool(name="persist", bufs=1) as pers,
            tc.tile_pool(name="qk", bufs=2) as qkp,
            tc.tile_pool(name="v", bufs=2) as vpool,
            tc.tile_pool(name="pt", bufs=2) as ptp,
            tc.tile_pool(name="sm", bufs=2) as smp,
            tc.tile_pool(name="rbb", bufs=2) as rbbp,
            tc.tile_pool(name="ot", bufs=2) as otp,
            tc.tile_pool(name="z", bufs=3) as zp,
            tc.tile_pool(name="psA", bufs=2, space="PSUM") as psA,
            tc.tile_pool(name="psS", bufs=1, space="PSUM") as psS,
            tc.tile_pool(name="psO", bufs=2, space="PSUM") as psO,
            tc.tile_pool(name="psD", bufs=1, space="PSUM") as psD,
            tc.tile_pool(name="psR", bufs=1, space="PSUM") as psR,
        ):
            # persistent loads; yT as 16 separate tiles (tile-granular dep
            # tracking: consumers of tp0's chunks mustn't wait on all of yT)
            yts, wqks, wvs, wos = {}, [], [], []
            for fi in range(4):
                wqks.append(pers.tile([128, 2 * E], bf16, tag=f"wqk{fi}",
                                      name=f"wqk{fi}"))
                wvs.append(pers.tile([128, E], bf16, tag=f"wv{fi}",
                                     name=f"wv{fi}"))
                wos.append(pers.tile([128, E], bf16, tag=f"wo{fi}",
                                     name=f"wo{fi}"))
                for tp in range(4):
                    yts[(fi, tp)] = pers.tile([128, 512], bf16,
                                              tag=f"yt{fi}_{tp}",
                                              name=f"yt{fi}_{tp}")
            for tp in range(4):
                for fi in range(4):
                    nc.sync.dma_start(
                        yts[(fi, tp)][:],
                        YT[fi * 128:(fi + 1) * 128, tp * 512:(tp + 1) * 512])
            for fi in range(4):
                nc.scalar.dma_start(wqks[fi][:], WQK[fi * 128:(fi + 1) * 128, :])
            for fi in range(4):
                nc.gpsimd.dma_start(wvs[fi][:], WV[fi * 128:(fi + 1) * 128, :])
            bqks = pers.tile([128, 4], f32, tag="bqk")
            nc.gpsimd.dma_start(bqks[:], BQK[:])
            # staircase selector for D-row matmuls: ones at col 3, so
            # stair[:, 3-k:7-k] has its ones in local column k (k in 0..3)
            stair = pers.tile([128, 8], bf16, tag="stair")
            nc.gpsimd.dma_start(stair[:], STAIR[:])
            # (ti, head-half) selector for the invD broadcast matmul:
            # sel4[g, ti*128+p] = 1 iff g == 2*ti + (p >= 64)
            sel4 = pers.tile([4, 256], bf16, tag="sel4")
            nc.gpsimd.dma_start(sel4[:], SEL4[:])
            beffb = pers.tile([128, E], f32, tag="beffb")
            nc.gpsimd.dma_start(
                beffb[:], BEFF[:].partition_broadcast(128).squeeze(1))
            for fi in range(4):
                nc.gpsimd.dma_start(wos[fi][:], WO[fi * 128:(fi + 1) * 128, :])

            for tp in range(4):  # t-pairs: 512 tokens each
                ptok0 = tp * 2 * MTOK
                # ---- QK^T projection: 8 r-tiles of (128, 512) ----
                qk = []
                for ri in range(8):
                    ps = psA.tile([128, 512], f32, tag="ps")
                    for fi in range(4):
                        nc.tensor.matmul(
                            ps[:],
                            wqks[fi][:, ri * 128:(ri + 1) * 128],
                            yts[(fi, tp)][:],
                            start=(fi == 0), stop=(fi == 3))
                    qt = qkp.tile([128, 512], bf16, tag=f"qk{ri}",
                                  name=f"qk{ri}_{tp}")
                    if ri < 4:  # Q rows carry the bias; K bias is droppable
                        nc.vector.tensor_scalar_add(qt[:], ps[:],
                                                    bqks[:, ri:ri + 1])
                    else:
                        nc.vector.tensor_copy(qt[:], ps[:])
                    qk.append(qt)
                # ---- V projection: token-major, 4 chunks of 128 tokens ----
                vts = []
                for sc in range(4):
                    psv = psA.tile([128, 512], f32, tag="ps")
                    for fi in range(4):
                        nc.tensor.matmul(
                            psv[:],
                            yts[(fi, tp)][:, sc * 128:(sc + 1) * 128],
                            wvs[fi][:],
                            start=(fi == 0), stop=(fi == 3))
                    vt = vpool.tile([128, E], bf16, tag=f"v{sc}",
                                    name=f"v{sc}_{tp}")
                    nc.scalar.copy(vt[:], psv[:])
                    vts.append(vt)
                # ---- attention per head-pair ----
                ot_all = {}
                for hp in range(4):
                    pts = {}
                    for ti in range(2):
                        for hh in range(2):
                            ho = hh * 64
                            pss = psS.tile([128, 512], f32, tag=f"pss{hh}",
                                           name=f"pss_{tp}_{hp}_{ti}_{hh}")
                            for sc in range(2):
                                nc.tensor.matmul(
                                    pss[:, sc * MTOK:(sc + 1) * MTOK],
                                    qk[4 + hp][ho:ho + 64,
                                               ti * MTOK + sc * 128:
                                               ti * MTOK + (sc + 1) * 128],
                                    qk[hp][ho:ho + 64,
                                           ti * MTOK:(ti + 1) * MTOK],
                                    start=True, stop=True,
                                    tile_position=(ho, 0),
                                    skip_group_check=True)
                            pt = ptp.tile([128, 512], bf16, tag=f"pt{ti}{hh}",
                                          name=f"pt_{tp}_{hp}_{ti}_{hh}")
                            nc.scalar.activation(pt[:], pss[:], Exp)
                            pts[(ti, hh)] = pt
                    # D rows: dall[k=(ti,hh), l] = sum_s exp(S^T[s, l])
                    dall = psD.tile([4, MTOK], f32, tag="dall",
                                    name=f"dall_{tp}_{hp}")
                    for ti in range(2):
                        for hh in range(2):
                            k = ti * 2 + hh
                            for sc in range(2):
                                nc.tensor.matmul(
                                    dall[:],
                                    stair[:, 3 - k:7 - k],
                                    pts[(ti, hh)][:, sc * MTOK:(sc + 1) * MTOK],
                                    start=(k == 0 and sc == 0),
                                    stop=(k == 3 and sc == 1),
                                    skip_group_check=True)
                    # AV, both heads concurrent via PE column groups
                    pso = psO.tile([128, 512], f32, tag="pso",
                                   name=f"pso_{tp}_{hp}")
                    for ti in range(2):
                        for hh in range(2):
                            h = hp * 2 + hh
                            for sc in range(2):
                                nc.tensor.matmul(
                                    pso[hh * 64:(hh + 1) * 64,
                                        ti * MTOK:(ti + 1) * MTOK],
                                    vts[2 * ti + sc][:, h * 64:(h + 1) * 64],
                                    pts[(ti, hh)][:, sc * MTOK:(sc + 1) * MTOK],
                                    start=(sc == 0), stop=(sc == 1),
                                    tile_position=(0, hh * 64),
                                    skip_group_check=True)
                    # batched 1/D (runs on DVE while AV matmuls stream)
                    invf = smp.tile([4, MTOK], f32, tag="invf",
                                    name=f"invf_{tp}_{hp}")
                    with nc.allow_low_precision(
                            reason="~18-bit 1/D for softmax; ~4e-6 rel"):
                        nc.vector.reciprocal_approx_fast(
                            out=invf[:], in_=dall[:])
                    invd = smp.tile([4, MTOK], bf16, tag="invd",
                                    name=f"invd_{tp}_{hp}")
                    nc.vector.tensor_copy(invd[:], invf[:])
                    # broadcast invD rows across partitions: PE outer product
                    rbps = psR.tile([128, 512], f32, tag="rb",
                                    name=f"rb_{tp}_{hp}")
                    for ti in range(2):
                        nc.tensor.matmul(
                            rbps[:, ti * MTOK:(ti + 1) * MTOK],
                            sel4[:, ti * 128:(ti + 1) * 128],
                            invd[:],
                            start=True, stop=True,
                            skip_group_check=True)
                    rbb = rbbp.tile([128, 512], f32, tag="rbb",
                                    name=f"rbb_{tp}_{hp}")
                    nc.scalar.copy(rbb[:], rbps[:])
                    ot = otp.tile([128, 512], bf16, tag=f"ot{hp}",
                                  name=f"ot{hp}_{tp}")
                    nc.vector.tensor_mul(ot[:], pso[:], rbb[:])
                    ot_all[hp] = ot
                # ---- out projection ----
                for ti in range(2):
                    t = tp * 2 + ti
                    tok0 = t * MTOK
                    for lc in range(2):
                        psz = psA.tile([128, E], f32, tag="ps",
                                       name=f"psz_{t}_{lc}")
                        for fi in range(4):
                            nc.tensor.matmul(
                                psz[:],
                                ot_all[fi][:, ti * MTOK + lc * 128:
                                           ti * MTOK + (lc + 1) * 128],
                                wos[fi][:],
                                start=(fi == 0), stop=(fi == 3))
                        zt = zp.tile([128, E], bf16, tag="zt",
                                     name=f"zt_{t}_{lc}")
                        nc.vector.tensor_add(zt[:], psz[:], beffb[:])
                        eng = nc.sync if lc == 0 else nc.scalar
                        eng.dma_start(
                            OUT[tok0 + lc * 128:tok0 + (lc + 1) * 128, :],
                            zt[:])

    # populate .instr bytes for extended-inst InstISA subclasses (the
    # custom-DVE reciprocal) — raw Bass doesn't run this pass itself
    mybir.codegen_inst_isa_subclasses(nc)
    if split_waits:
        _split_multiwaits(nc, mybir)
    return nc


def _host_prep(x, in_proj_w, in_proj_b, out_proj_w, out_proj_b):
    x = np.asarray(x, dtype=np.float32)
    in_proj_w = np.asarray(in_proj_w, dtype=np.float32)
    in_proj_b = np.asarray(in_proj_b, dtype=np.float32)
    out_proj_w = np.asarray(out_proj_w, dtype=np.float32)
    out_proj_b = np.asarray(out_proj_b, dtype=np.float32)

    # weights (replicated); fold the 1/sqrt(hd)=1/8 score scale into q rows
    wq = in_proj_w[:E] / 8.0
    wk = in_proj_w[E:2 * E]
    wv = in_proj_w[2 * E:]
    wqk = np.concatenate([wq, wk], 0).T.copy().astype(BF)        # (512, 1024)
    wvT = wv.T.copy().astype(BF)                                 # (512, 512)
    wo = out_proj_w.T.copy().astype(BF)                          # (512, 512)
    # only Q needs its bias (softmax is invariant to the K bias)
    bqk = (in_proj_b[:E] / 8.0).reshape(4, 128).T.copy().astype(np.float32)
    beff = (out_proj_b + out_proj_w @ in_proj_b[2 * E:]).reshape(1, E)
    beff = beff.astype(np.float32)
    stair = np.zeros((128, 8), dtype=BF)
    stair[:, 3] = 1
    sel4 = np.zeros((4, 256), dtype=BF)
    for ti in range(2):
        sel4[2 * ti, ti * 128:ti * 128 + 64] = 1.0
        sel4[2 * ti + 1, ti * 128 + 64:ti * 128 + 128] = 1.0

    # per-core token matrices: yT[f=(a,w1,w2), col=(tl, b, i, j)]
    # channel c = 64*cc + 8*tl + a  (t = 8*cc + tl)
    xv = x.reshape(B, HS, WIN, HS, WIN, NCORES, TL, WIN)  # b i w1 j w2 cc tl a
    yts = []
    for cc in range(NCORES):
        yt = xv[:, :, :, :, :, cc].transpose(6, 2, 4, 5, 0, 1, 3)
        yts.append(np.ascontiguousarray(yt).reshape(E, TOK).astype(BF))
    consts = {"wqk": wqk, "wv": wvT, "wo": wo, "bqk": bqk, "beff": beff,
              "stair": stair, "sel4": sel4}
    return yts, consts


def kernel(x, in_proj_w, in_proj_b, out_proj_w, out_proj_b,
           window_size=8, nhead=8, **_unused):
    from concourse.bass_utils import run_bass_kernel_spmd

    yts, consts = _host_prep(
        x, in_proj_w, in_proj_b, out_proj_w, out_proj_b)

    if "nc" not in _cache:
        _cache["nc"] = _build_module()
    nc = _cache["nc"]

    in_maps = [{"yT": yts[cc], **consts} for cc in range(NCORES)]
    res = run_bass_kernel_spmd(nc, in_maps, core_ids=list(range(NCORES)))

    out = np.empty((B, N, E), dtype=np.float32)
    ov = out.reshape(B, HS, WIN, HS, WIN, E)  # b i w1 j w2 e
    for cc in range(NCORES):
        z = np.asarray(res.results[cc]["o"]).astype(np.float32)
        z = z.reshape(TL, B, HS, HS, E)  # tl b i j e
        # t = 8*cc + tl -> w1 = cc, w2 = tl
        ov[:, :, cc, :, :, :] = z.transpose(1, 2, 3, 0, 4)
    return out


# revision 29
# speedup vs baseline: 1.1010x; 1.1010x over previous
"""Local2DAttention TRN2 kernel (nn_Local2DAttention_79207786873330).

Math (faithful to the reference's torch-bug semantics):
  x (16, 1024, 512) is window-blocked into M=256 "windows" (b, i, j) of 8x8
  spatial positions. A plain row-major reshape of each (E, 8, 8) block into
  (64, 512) scrambles channels/spatial into 64 tokens per window:
      y[m, t, e] = x[b, (i*8+w1)*32 + j*8 + w2, 8t+a],  e = a*64 + w1*8 + w2
  nn.MultiheadAttention (batch_first=False) then attends over the M=256 axis
  with the 64 t-positions as batch and 8 heads:
      per (t, h): S = Q K^T / 8 over 256x256, softmax, O = P V.

Sharding: the 64 t-positions split 8 per core (t = 8*cc + tl). Attention,
projections and output assembly are fully independent per t -> zero
cross-core communication. Weights are replicated.

Device pipeline per core (bf16 matmul operands, fp32 accumulation):
  yT (512, 2048)      - host-permuted token matrix, f-major (e x tokens)
  QK^T proj: PROJ^T[r, tok] = Wqk^T.T @ yT  (q rows pre-scaled by 1/8;
             K bias dropped - softmax is invariant to it)
  V    proj: V[tok, 512]  = yT.T @ Wv  (plain 64-col heads, no ones col)
  per (t-pair tp, head-pair hp):
      scores S^T = K^T.T @ Q^T -> exp (ACT, ->bf16) per (ti, hh)
      D rows via PE "staircase" matmuls: dall[k=(ti,hh), l] = sum_s exp
      AV col-group packed: pso[hh*64+d, ti*256+l] (2 heads concurrent)
      one reciprocal_approx_fast (4, 256) -> invD (f32r)
      PE broadcast invD -> rbps (128, 512), ACT evict -> SBUF
      one DVE mul: ot = pso * rbb  (normalized O^T, bf16)
  out proj: Z = O^T.T @ Wout^T + b_eff  (b_eff = b_out + Wout @ b_v, host)
  output stored bf16, upcast on host.
"""
import sys
sys.path.insert(0, '/opt/trn_rl_repo')
import numpy as np
import ml_dtypes

BF = ml_dtypes.bfloat16

# problem constants (hardcoded per contract)
B, N, E = 16, 1024, 512
WIN = 8          # window_size
HS = 4           # hS = S // W,  S = 32
NH = 8           # heads
HD = 64          # head dim
NCORES = 8
TL = 8           # t-values per core
MTOK = 256       # windows (= B*HS*HS) = tokens per t
TOK = TL * MTOK  # tokens per core

_cache = {}


def _split_multiwaits(nc, mybir, limit=1):
    """This toolchain's walrus encodes at most one semaphore wait per
    instruction; hoist excess waits into preceding NoOps on the same engine."""
    n_split = 0
    for f in nc.m.functions:
        for blk in f.blocks:
            insts = blk.instructions
            out = []
            for inst in insts:
                si = inst.sync_info
                waits = list(si.on_wait) if (si is not None and si.on_wait) else []
                if len(waits) > limit:
                    excess, keep = waits[:-limit], waits[-limit:]
                    for w in excess:
                        nop = mybir.InstNoOp(
                            name=f"{inst.name}-wsplit{n_split}",
                            engine=inst.engine,
                            ins=[], outs=[],
                            sync_info=mybir.SyncInfo(on_wait=[w], on_update=[]),
                        )
                        out.append(nop)
                        n_split += 1
                    inst.sync_info = mybir.SyncInfo(
                        on_wait=keep, on_update=list(si.on_update or []))
                out.append(inst)
            if n_split:
                insts.clear()
                insts.extend(out)
    return n_split


def _build_module(split_waits=True):
    import concourse.bass as bass
    import concourse.mybir as mybir
    from concourse import tile

    f32 = mybir.dt.float32
    f32r = mybir.dt.float32r
    bf16 = mybir.dt.bfloat16
    Exp = mybir.ActivationFunctionType.Exp

    nc = bass.Bass()
    YT = nc.dram_tensor("yT", [E, TOK], bf16, kind="ExternalInput")
    WQK = nc.dram_tensor("wqk", [E, 2 * E], bf16, kind="ExternalInput")
    WV = nc.dram_tensor("wv", [E, E], bf16, kind="ExternalInput")
    WO = nc.dram_tensor("wo", [E, E], bf16, kind="ExternalInput")
    BQK = nc.dram_tensor("bqk", [128, 4], f32, kind="ExternalInput")
    BEFF = nc.dram_tensor("beff", [1, E], f32, kind="ExternalInput")
    STAIR = nc.dram_tensor("stair", [128, 8], bf16, kind="ExternalInput")
    SEL4 = nc.dram_tensor("sel4", [4, 256], bf16, kind="ExternalInput")
    OUT = nc.dram_tensor("o", [TOK, E], bf16, kind="ExternalOutput")

    with tile.TileContext(nc) as tc:
        with (
            tc.tile_pool(name="persist", bufs=1) as pers,
            tc.tile_pool(name="qk", bufs=2) as qkp,
            tc.tile_pool(name="v", bufs=2) as vpool,
            tc.tile_pool(name="pt", bufs=2) as ptp,
            tc.tile_pool(name="sm", bufs=2) as smp,
            tc.tile_pool(name="rbb", bufs=2) as rbbp,
            tc.tile_pool(name="ot", bufs=2) as otp,
            tc.tile_pool(name="z", bufs=3) as zp,
            tc.tile_pool(name="psA", bufs=2, space="PSUM") as psA,
            tc.tile_pool(name="psS", bufs=1, space="PSUM") as psS,
            tc.tile_pool(name="psO", bufs=2, space="PSUM") as psO,
            tc.tile_pool(name="psD", bufs=1, space="PSUM") as psD,
            tc.tile_pool(name="psR", bufs=1, space="PSUM") as psR,
        ):
            # persistent loads; yT as 16 separate tiles (tile-granular dep
            # tracking: consumers of tp0's chunks mustn't wait on all of yT)
            yts, wqks, wvs, wos = {}, [], [], []
            for fi in range(4):
                wqks.append(pers.tile([128, 2 * E], bf16, tag=f"wqk{fi}",
                                      name=f"wqk{fi}"))
                wvs.append(pers.tile([128, E], bf16, tag=f"wv{fi}",
                                     name=f"wv{fi}"))
                wos.append(pers.tile([128, E], bf16, tag=f"wo{fi}",
                                     name=f"wo{fi}"))
                for tp in range(4):
                    yts[(fi, tp)] = pers.tile([128, 512], bf16,
                                              tag=f"yt{fi}_{tp}",
                                              name=f"yt{fi}_{tp}")
            for tp in range(4):
                for fi in range(4):
                    nc.sync.dma_start(
                        yts[(fi, tp)][:],
                        YT[fi * 128:(fi + 1) * 128, tp * 512:(tp + 1) * 512])
            for fi in range(4):
                nc.scalar.dma_start(wqks[fi][:], WQK[fi * 128:(fi + 1) * 128, :])
            for fi in range(4):
                nc.gpsimd.dma_start(wvs[fi][:], WV[fi * 128:(fi + 1) * 128, :])
            bqks = pers.tile([128, 4], f32, tag="bqk")
            nc.gpsimd.dma_start(bqks[:], BQK[:])
            # staircase selector for D-row matmuls: ones at col 3, so
            # stair[:, 3-k:7-k] has its ones in local column k (k in 0..3)
            stair = pers.tile([128, 8], bf16, tag="stair")
            nc.gpsimd.dma_start(stair[:], STAIR[:])
            # (ti, head-half) selector for the invD broadcast matmul:
            # sel4[g, ti*128+p] = 1 iff g == 2*ti + (p >= 64)
            sel4 = pers.tile([4, 256], bf16, tag="sel4")
            nc.gpsimd.dma_start(sel4[:], SEL4[:])
            beffb = pers.tile([128, E], f32, tag="beffb")
            nc.gpsimd.dma_start(
                beffb[:], BEFF[:].partition_broadcast(128).squeeze(1))
            for fi in range(4):
                nc.gpsimd.dma_start(wos[fi][:], WO[fi * 128:(fi + 1) * 128, :])

            for tp in range(4):  # t-pairs: 512 tokens each
                ptok0 = tp * 2 * MTOK
                # ---- V projection first (smaller weight load -> earlier
                # start on tp0): token-major, 4 chunks of 128 tokens ----
                vts = []
                for sc in range(4):
                    psv = psA.tile([128, 512], f32, tag="ps")
                    for fi in range(4):
                        nc.tensor.matmul(
                            psv[:],
                            yts[(fi, tp)][:, sc * 128:(sc + 1) * 128],
                            wvs[fi][:],
                            start=(fi == 0), stop=(fi == 3))
                    vt = vpool.tile([128, E], bf16, tag=f"v{sc}",
                                    name=f"v{sc}_{tp}")
                    nc.scalar.copy(vt[:], psv[:])
                    vts.append(vt)
                # ---- QK^T projection: 8 r-tiles of (128, 512) ----
                qk = []
                for ri in range(8):
                    ps = psA.tile([128, 512], f32, tag="ps")
                    for fi in range(4):
                        nc.tensor.matmul(
                            ps[:],
                            wqks[fi][:, ri * 128:(ri + 1) * 128],
                            yts[(fi, tp)][:],
                            start=(fi == 0), stop=(fi == 3))
                    qt = qkp.tile([128, 512], bf16, tag=f"qk{ri}",
                                  name=f"qk{ri}_{tp}")
                    if ri < 4:  # Q rows carry the bias; K bias is droppable
                        nc.vector.tensor_scalar_add(qt[:], ps[:],
                                                    bqks[:, ri:ri + 1])
                    else:
                        nc.vector.tensor_copy(qt[:], ps[:])
                    qk.append(qt)
                # ---- attention per head-pair ----
                ot_all = {}
                for hp in range(4):
                    pts = {}
                    for ti in range(2):
                        psss = {}
                        for hh in range(2):
                            psss[hh] = psS.tile([128, 512], f32,
                                                tag=f"pss{hh}",
                                                name=f"pss_{tp}_{hp}_{ti}_{hh}")
                        # sc-major: adjacent MMs use disjoint PE row groups
                        # (hh0 rows 0-64, hh1 rows 64-128) -> they overlap
                        for sc in range(2):
                            for hh in range(2):
                                ho = hh * 64
                                nc.tensor.matmul(
                                    psss[hh][:, sc * MTOK:(sc + 1) * MTOK],
                                    qk[4 + hp][ho:ho + 64,
                                               ti * MTOK + sc * 128:
                                               ti * MTOK + (sc + 1) * 128],
                                    qk[hp][ho:ho + 64,
                                           ti * MTOK:(ti + 1) * MTOK],
                                    start=True, stop=True,
                                    tile_position=(ho, 0),
                                    skip_group_check=True)
                        for hh in range(2):
                            pt = ptp.tile([128, 512], bf16, tag=f"pt{ti}{hh}",
                                          name=f"pt_{tp}_{hp}_{ti}_{hh}")
                            nc.scalar.activation(pt[:], psss[hh][:], Exp)
                            pts[(ti, hh)] = pt
                    # D rows: dall[k=(ti,hh), l] = sum_s exp(S^T[s, l])
                    dall = psD.tile([4, MTOK], f32, tag="dall",
                                    name=f"dall_{tp}_{hp}")
                    for ti in range(2):
                        for hh in range(2):
                            k = ti * 2 + hh
                            for sc in range(2):
                                nc.tensor.matmul(
                                    dall[:],
                                    stair[:, 3 - k:7 - k],
                                    pts[(ti, hh)][:, sc * MTOK:(sc + 1) * MTOK],
                                    start=(k == 0 and sc == 0),
                                    stop=(k == 3 and sc == 1),
                                    skip_group_check=True)
                    # AV, both heads concurrent via PE column groups
                    pso = psO.tile([128, 512], f32, tag="pso",
                                   name=f"pso_{tp}_{hp}")
                    for ti in range(2):
                        for sc in range(2):
                            for hh in range(2):
                                h = hp * 2 + hh
                                nc.tensor.matmul(
                                    pso[hh * 64:(hh + 1) * 64,
                                        ti * MTOK:(ti + 1) * MTOK],
                                    vts[2 * ti + sc][:, h * 64:(h + 1) * 64],
                                    pts[(ti, hh)][:, sc * MTOK:(sc + 1) * MTOK],
                                    start=(sc == 0), stop=(sc == 1),
                                    tile_position=(0, hh * 64),
                                    skip_group_check=True)
                    # batched 1/D (runs on DVE while AV matmuls stream)
                    invf = smp.tile([4, MTOK], f32, tag="invf",
                                    name=f"invf_{tp}_{hp}")
                    with nc.allow_low_precision(
                            reason="~18-bit 1/D for softmax; ~4e-6 rel"):
                        nc.vector.reciprocal_approx_fast(
                            out=invf[:], in_=dall[:])
                    invd = smp.tile([4, MTOK], bf16, tag="invd",
                                    name=f"invd_{tp}_{hp}")
                    nc.vector.tensor_copy(invd[:], invf[:])
                    # broadcast invD rows across partitions: PE outer product
                    rbps = psR.tile([128, 512], f32, tag="rb",
                                    name=f"rb_{tp}_{hp}")
                    for ti in range(2):
                        nc.tensor.matmul(
                            rbps[:, ti * MTOK:(ti + 1) * MTOK],
                            sel4[:, ti * 128:(ti + 1) * 128],
                            invd[:],
                            start=True, stop=True,
                            skip_group_check=True)
                    rbb = rbbp.tile([128, 512], f32, tag="rbb",
                                    name=f"rbb_{tp}_{hp}")
                    nc.scalar.copy(rbb[:], rbps[:])
                    ot = otp.tile([128, 512], bf16, tag=f"ot{hp}",
                                  name=f"ot{hp}_{tp}")
                    nc.vector.tensor_mul(ot[:], pso[:], rbb[:])
                    ot_all[hp] = ot
                # ---- out projection ----
                for ti in range(2):
                    t = tp * 2 + ti
                    tok0 = t * MTOK
                    for lc in range(2):
                        # reuse the scores banks (idle during out-proj) so
                        # next tp's projections don't WAR-stall on psA
                        psz = psS.tile([128, E], f32, tag=f"pss{lc}",
                                       name=f"psz_{t}_{lc}")
                        for fi in range(4):
                            nc.tensor.matmul(
                                psz[:],
                                ot_all[fi][:, ti * MTOK + lc * 128:
                                           ti * MTOK + (lc + 1) * 128],
                                wos[fi][:],
                                start=(fi == 0), stop=(fi == 3))
                        zt = zp.tile([128, E], bf16, tag="zt",
                                     name=f"zt_{t}_{lc}")
                        nc.vector.tensor_add(zt[:], psz[:], beffb[:])
                        eng = nc.sync if lc == 0 else nc.scalar
                        eng.dma_start(
                            OUT[tok0 + lc * 128:tok0 + (lc + 1) * 128, :],
                            zt[:])

    # populate .instr bytes for extended-inst InstISA subclasses (the
    # custom-DVE reciprocal) — raw Bass doesn't run this pass itself
    mybir.codegen_inst_isa_subclasses(nc)
    if split_waits:
        _split_multiwaits(nc, mybir)
    return nc


def _host_prep(x, in_proj_w, in_proj_b, out_proj_w, out_proj_b):
    x = np.asarray(x, dtype=np.float32)
    in_proj_w = np.asarray(in_proj_w, dtype=np.float32)
    in_proj_b = np.asarray(in_proj_b, dtype=np.float32)
    out_proj_w = np.asarray(out_proj_w, dtype=np.float32)
    out_proj_b = np.asarray(out_proj_b, dtype=np.float32)

    # weights (replicated); fold the 1/sqrt(hd)=1/8 score scale into q rows
    wq = in_proj_w[:E] / 8.0
    wk = in_proj_w[E:2 * E]
    wv = in_proj_w[2 * E:]
    wqk = np.concatenate([wq, wk], 0).T.copy().astype(BF)        # (512, 1024)
    wvT = wv.T.copy().astype(BF)                                 # (512, 512)
    wo = out_proj_w.T.copy().astype(BF)                          # (512, 512)
    # only Q needs its bias (softmax is invariant to the K bias)
    bqk = (in_proj_b[:E] / 8.0).reshape(4, 128).T.copy().astype(np.float32)
    beff = (out_proj_b + out_proj_w @ in_proj_b[2 * E:]).reshape(1, E)
    beff = beff.astype(np.float32)
    stair = np.zeros((128, 8), dtype=BF)
    stair[:, 3] = 1
    sel4 = np.zeros((4, 256), dtype=BF)
    for ti in range(2):
        sel4[2 * ti, ti * 128:ti * 128 + 64] = 1.0
        sel4[2 * ti + 1, ti * 128 + 64:ti * 128 + 128] = 1.0

    # per-core token matrices: yT[f=(a,w1,w2), col=(tl, b, i, j)]
    # channel c = 64*cc + 8*tl + a  (t = 8*cc + tl)
    xv = x.reshape(B, HS, WIN, HS, WIN, NCORES, TL, WIN)  # b i w1 j w2 cc tl a
    yts = []
    for cc in range(NCORES):
        yt = xv[:, :, :, :, :, cc].transpose(6, 2, 4, 5, 0, 1, 3)
        yts.append(np.ascontiguousarray(yt).reshape(E, TOK).astype(BF))
    consts = {"wqk": wqk, "wv": wvT, "wo": wo, "bqk": bqk, "beff": beff,
              "stair": stair, "sel4": sel4}
    return yts, consts


def kernel(x, in_proj_w, in_proj_b, out_proj_w, out_proj_b,
           window_size=8, nhead=8, **_unused):
    from concourse.bass_utils import run_bass_kernel_spmd

    yts, consts = _host_prep(
        x, in_proj_w, in_proj_b, out_proj_w, out_proj_b)

    if "nc" not in _cache:
        _cache["nc"] = _build_module()
    nc = _cache["nc"]

    in_maps = [{"yT": yts[cc], **consts} for cc in range(NCORES)]
    res = run_bass_kernel_spmd(nc, in_maps, core_ids=list(range(NCORES)))

    out = np.empty((B, N, E), dtype=np.float32)
    ov = out.reshape(B, HS, WIN, HS, WIN, E)  # b i w1 j w2 e
    for cc in range(NCORES):
        z = np.asarray(res.results[cc]["o"]).astype(np.float32)
        z = z.reshape(TL, B, HS, HS, E)  # tl b i j e
        # t = 8*cc + tl -> w1 = cc, w2 = tl
        ov[:, :, cc, :, :, :] = z.transpose(1, 2, 3, 0, 4)
    return out
